# revision 23
# baseline (speedup 1.0000x reference)
"""BEV detection loss on 8 Trainium2 NeuronCores.

Strategy (data-parallel over batch, one batch element per core):
  - The loss touches cls_logits / box_preds ONLY at positive cells (cells
    that won a GT box in the first-come-wins scatter assignment, <= 64 per
    batch element).  Only obj_logits needs a full scan (sum of softplus
    over all 262144 cells per batch element).
  - Host does the (tiny, 64-box) scatter assignment per batch element with
    bit-identical float32 index math, gathers the <=64 positive rows, and
    packs them together with the 262144 obj logits into ONE [128, 2084]
    input tensor per core.  The tensor streams in as 4 DMA chunks split
    across the two HWDGE queues (sync + scalar engines) so transfers
    overlap.
  - Each core scans its obj logits computing sum(softplus(x)) as
    ln(1+exp(x)) on the ACT engine (the only engine with transcendentals;
    the toolchain's ACT table overlay has no softplus, so exp+ln it is).
    All exp/ln resolve to the combined natural_log_exp_and_others table
    set, giving exactly ONE table load, prefetched before data arrives.
    The vector engine does all reductions, smooth-L1, and softplus(o) =
    o + softplus(-o); cross-entropy needs one small exp and ln.
  - Host combines per-core partials with the globally-consistent
    pos_weight and means (all in float32, matching the reference).
"""

import sys

import numpy as np

sys.path.insert(0, "/opt/trn_rl_repo")

import concourse.bacc as bacc  # noqa: E402
import concourse.mybir as mybir  # noqa: E402
import concourse.tile as tile  # noqa: E402
from concourse.bass_utils import run_bass_kernel_spmd  # noqa: E402

# BEV grid constants (must match the reference)
X_MIN = np.float32(-51.2)
X_MAX = np.float32(51.2)
Y_MIN = np.float32(-51.2)
Y_MAX = np.float32(51.2)
RES = np.float32(0.2)
BEV_W = 512
BEV_H = 512
NUM_CELLS = BEV_W * BEV_H  # 262144
CLS_WEIGHT = np.float32(1.0)
BOX_WEIGHT = np.float32(1.0)

N_CORES = 8
P_DIM = 128
COLS = NUM_CELLS // P_DIM  # 2048
NMAX = 64
C = 10
D = 7
# packed positives layout (within their 36-col block):
# 0 obj | 1..10 cls | 11..20 onehot | 21..27 box_pred | 28..34 box_tgt | 35 pad
POS_W = 36
POS_OFF = COLS  # positives at the tail: cols 2048..2083
IN_W = COLS + POS_W  # 2084
# DMA chunks: tiny positives block first (sync), then obj chunks alternating
# between the sync HWDGE and gpsimd SWDGE queues
DMA_RANGES = [(0, 512), (512, 1024), (1024, 1536), (1536, 2048)]
# ACT compute chunks over the obj columns (1024-col ACT ops are ~1.5x more
# efficient per element than 512-col ones)
SP_RANGES = [(0, 1024), (1024, 2048)]
# output layout [128, 5]: col 0 = obj softplus partition sums;
# cols 1..4 (rows 0..63) = softplus(-o), softplus(o), ce, box row sums
OUT_W = 5

_CACHE = {}


def _build_program():
    f32 = mybir.dt.float32
    AF = mybir.ActivationFunctionType
    AX = mybir.AxisListType

    nc = bacc.Bacc("TRN2", debug=False, target_bir_lowering=False, num_devices=N_CORES)
    in_all = nc.dram_tensor("in_all", [P_DIM, IN_W], f32, kind="ExternalInput").ap()
    out_all = nc.dram_tensor("out_all", [P_DIM, OUT_W], f32, kind="ExternalOutput").ap()

    with tile.TileContext(nc) as tc:
        with (
            tc.tile_pool(name="big", bufs=1) as big,
            tc.tile_pool(name="small", bufs=1) as small,
        ):
            x = small.tile([P_DIM, IN_W], f32)
            # tiny positives block first on the gpsimd queue (which only gates
            # the second half of the scan): positives data is ready ~1us after
            # block entry, so the positive-cell pipeline overlaps the obj scan
            nc.gpsimd.dma_start(
                out=x[0:NMAX, POS_OFF:IN_W], in_=in_all[0:NMAX, POS_OFF:IN_W]
            )
            for i, (lo, hi) in enumerate(DMA_RANGES):
                eng = nc.sync if i % 2 == 0 else nc.gpsimd
                eng.dma_start(out=x[:, lo:hi], in_=in_all[:, lo:hi])

            # Data-independent warmup ACT: forces the exp/ln table load to be
            # placed at block start (before any DMA wait) so it overlaps the
            # input transfers.  scale=0.0 means the input is never actually
            # read (out = exp(0)).
            warm = small.tile([P_DIM, 1], f32)
            nc.scalar.activation(warm[:], warm[:], AF.Exp, scale=0.0)

            out = small.tile([P_DIM, OUT_W], f32)
            nc.vector.memset(out[:], 0.0)

            # positives views (rows 0..63)
            o = x[0:NMAX, POS_OFF : POS_OFF + 1]
            cls = x[0:NMAX, POS_OFF + 1 : POS_OFF + 1 + C]
            oh = x[0:NMAX, POS_OFF + 11 : POS_OFF + 11 + C]
            bp = x[0:NMAX, POS_OFF + 21 : POS_OFF + 21 + D]
            bt = x[0:NMAX, POS_OFF + 28 : POS_OFF + 28 + D]

            # ---- positives: runs entirely during the obj scan ----
            m = small.tile([NMAX, 1], f32)
            nc.vector.reduce_max(m[:], cls, axis=AX.X)
            sh = small.tile([NMAX, C], f32)
            nc.vector.tensor_scalar_sub(sh[:], cls, m[:])
            e1 = small.tile([NMAX, 1], f32)
            nc.scalar.activation(e1[:], o, AF.Exp, scale=-1.0)
            nc.scalar.activation(out[0:NMAX, 1:2], e1[:], AF.Ln, bias=1.0)
            esh = small.tile([NMAX, C], f32)
            esum = small.tile([NMAX, 1], f32)
            nc.scalar.activation(esh[:], sh[:], AF.Exp, accum_out=esum[:])
            lse = small.tile([NMAX, 1], f32)
            nc.scalar.activation(lse[:], esum[:], AF.Ln)

            # softplus(o) = o + softplus(-o)
            nc.vector.tensor_add(out[0:NMAX, 2:3], o, out[0:NMAX, 1:2])
            prod = small.tile([NMAX, C], f32)
            nc.vector.tensor_mul(prod[:], cls, oh)
            xl = small.tile([NMAX, 1], f32)
            nc.vector.reduce_sum(xl[:], prod[:], axis=AX.X)
            mlse = small.tile([NMAX, 1], f32)
            nc.vector.tensor_add(mlse[:], m[:], lse[:])
            nc.vector.tensor_sub(out[0:NMAX, 3:4], mlse[:], xl[:])

            d_ = small.tile([NMAX, D], f32)
            nc.vector.tensor_sub(d_[:], bp, bt)
            nd = small.tile([NMAX, D], f32)
            nc.vector.tensor_scalar_mul(nd[:], d_[:], -1.0)
            ad = small.tile([NMAX, D], f32)
            nc.vector.tensor_max(ad[:], d_[:], nd[:])
            mn = small.tile([NMAX, D], f32)
            nc.vector.tensor_scalar_min(mn[:], ad[:], 1.0)
            mnsq = small.tile([NMAX, D], f32)
            nc.vector.tensor_mul(mnsq[:], mn[:], mn[:])
            hm = small.tile([NMAX, D], f32)
            nc.vector.tensor_scalar_mul(hm[:], mnsq[:], 0.5)
            admn = small.tile([NMAX, D], f32)
            nc.vector.tensor_sub(admn[:], ad[:], mn[:])
            sl1 = small.tile([NMAX, D], f32)
            nc.vector.tensor_add(sl1[:], hm[:], admn[:])
            nc.vector.reduce_sum(out[0:NMAX, 4:5], sl1[:], axis=AX.X)

            # ---- obj scan: sum softplus(x) = ln(1+exp(x)) per partition ----
            nchunk = len(SP_RANGES)
            sums = small.tile([P_DIM, nchunk], f32)
            for t, (lo, hi) in enumerate(SP_RANGES):
                w = hi - lo
                last = t == nchunk - 1
                e = big.tile([P_DIM, w], f32, tag=f"e{t}")
                nc.scalar.activation(e[:], x[:, lo:hi], AF.Exp)
                sp = big.tile([P_DIM, w], f32, tag=f"sp{t}")
                if last:
                    # fused accumulation: no trailing vector reduce on the
                    # critical path
                    nc.scalar.activation(
                        sp[:], e[:], AF.Ln, bias=1.0,
                        accum_out=sums[:, t : t + 1],
                    )
                else:
                    nc.scalar.activation(sp[:], e[:], AF.Ln, bias=1.0)
                    nc.vector.reduce_sum(sums[:, t : t + 1], sp[:], axis=AX.X)
            nc.vector.reduce_sum(out[:, 0:1], sums[:], axis=AX.X)

            nc.sync.dma_start(out=out_all[:], in_=out[:])

    # Finalize with activation tables restricted so exp and ln resolve to
    # the combined natural_log_exp_and_others set: one ACT table load for
    # the whole kernel instead of one per exp<->ln transition.
    orig_get = bacc.get_activation_tables
    AFT = mybir.ActivationFunctionType

    def _combined_tables(arch):
        t = orig_get(arch)
        for name, fns in list(t.items()):
            if name != "natural_log_exp_and_others" and (
                AFT.Exp in fns or AFT.Ln in fns
            ):
                t[name] = {f for f in fns if f not in (AFT.Exp, AFT.Ln)}
        return t

    bacc.get_activation_tables = _combined_tables
    try:
        nc.finalize()
    finally:
        bacc.get_activation_tables = orig_get
    return nc


def get_program():
    if "nc" not in _CACHE:
        _CACHE["nc"] = _build_program()
    return _CACHE["nc"]


def _assign_and_pack(cls_logits, obj_logits, box_preds, gt_boxes, gt_labels, gt_masks):
    """Host-side first-come-wins assignment; returns per-core packed
    positives [NMAX, POS_W] and per-core positive counts."""
    B, N = gt_labels.shape
    gb = np.asarray(gt_boxes, dtype=np.float32)
    x = gb[..., 0]
    y = gb[..., 1]
    in_b = (x >= X_MIN) & (x <= X_MAX) & (y >= Y_MIN) & (y <= Y_MAX)
    gx = np.clip(np.floor((x - X_MIN) / RES).astype(np.int32), 0, BEV_W - 1)
    gy = np.clip(np.floor((y - Y_MIN) / RES).astype(np.int32), 0, BEV_H - 1)
    idx = gy * BEV_W + gx  # [B, N]
    valid = (
        (np.asarray(gt_masks, dtype=np.float32) > 0.5)
        & (np.asarray(gt_labels) >= 0)
        & in_b
    )

    packs = np.zeros((B, NMAX, POS_W), np.float32)
    counts = []
    for b in range(B):
        seen = set()
        k = 0
        for n in range(N):
            if not valid[b, n]:
                continue
            cell = int(idx[b, n])
            if cell in seen:
                continue
            seen.add(cell)
            packs[b, k, 0] = obj_logits[b, cell]
            packs[b, k, 1 : 1 + C] = cls_logits[b, cell]
            packs[b, k, 11 + int(gt_labels[b, n])] = 1.0
            packs[b, k, 21 : 21 + D] = box_preds[b, cell]
            packs[b, k, 28 : 28 + D] = gb[b, n]
            k += 1
        counts.append(k)
    return packs, counts


def _combine(results, counts):
    """Host-side final reduction of per-core partials (float32 throughout)."""
    f32 = np.float32
    s_all = f32(0.0)
    s_neg = f32(0.0)
    s_pos = f32(0.0)
    s_ce = f32(0.0)
    s_box = f32(0.0)
    total_pos = 0
    for c, res in enumerate(results):
        out = res["out_all"].astype(np.float32)
        s_all += out[:, 0].sum(dtype=np.float32)
        pb = counts[c]
        total_pos += pb
        if pb:
            s_neg += out[:pb, 1].sum(dtype=np.float32)
            s_pos += out[:pb, 2].sum(dtype=np.float32)
            s_ce += out[:pb, 3].sum(dtype=np.float32)
            s_box += out[:pb, 4].sum(dtype=np.float32)

    M = f32(N_CORES * NUM_CELLS)
    positive = f32(total_pos)
    negatives = M - positive
    pos_weight = np.maximum(f32(1.0), negatives / (positive + f32(1e-6)))

    obj_loss = (s_all + pos_weight * s_neg - s_pos) / M
    if total_pos > 0:
        cls_loss = s_ce / np.maximum(positive, f32(1.0))
        box_loss = s_box / np.maximum(positive * f32(D), f32(1.0))
    else:
        cls_loss = f32(0.0)
        box_loss = f32(0.0)
    total = obj_loss + CLS_WEIGHT * cls_loss + BOX_WEIGHT * box_loss
    return np.array([total, cls_loss, box_loss, obj_loss], dtype=np.float32)


def _make_in_maps(obj_logits, packs):
    in_maps = []
    for b in range(N_CORES):
        buf = np.zeros((P_DIM, IN_W), np.float32)
        buf[:, :COLS] = np.asarray(obj_logits[b], dtype=np.float32).reshape(P_DIM, COLS)
        buf[0:NMAX, POS_OFF : POS_OFF + POS_W] = packs[b]
        in_maps.append({"in_all": buf})
    return in_maps


def kernel(cls_logits, obj_logits, box_preds, gt_boxes, gt_labels, gt_masks):
    cls_logits = np.asarray(cls_logits)
    obj_logits = np.asarray(obj_logits)
    box_preds = np.asarray(box_preds)
    B = obj_logits.shape[0]
    assert B == N_CORES, f"expected batch {N_CORES}, got {B}"

    packs, counts = _assign_and_pack(
        cls_logits, obj_logits, box_preds, gt_boxes, gt_labels, gt_masks
    )

    nc = get_program()
    in_maps = _make_in_maps(obj_logits, packs)
    res = run_bass_kernel_spmd(nc, in_maps, list(range(N_CORES))).results
    return _combine(res, counts)
